# revision 31
# baseline (speedup 1.0000x reference)
"""MultiHeadAttention Trainium2 kernel.

Sharding: B=2 batches x H=16 heads = 32 (b,h) pairs -> 4 heads per core.
Cores 0-3 handle batch 0 (heads 4c..4c+3), cores 4-7 batch 1.
Each core computes q/k/v projections for its head slice, transposed-scores
attention, and a partial output projection (sum over its heads of
o_h @ Wo[h-slice]).  Host sums the 4 partials per batch and adds bo.

Projection inputs/weights are bf16 (halves input DMA); scores operands
(qT/kT), oTn and Wo are float32r (rounded fp32, 1 cycle/row on PE at N>=256);
post-softmax probabilities and V are bf16.  Softmax skips max-subtraction
(scores ~ N(0,1); exp cannot overflow) and is computed in the transposed
layout PT[sk, sq] = exp(scores^T) * mask^T; denominators come from a
ones-column appended to V in the P@V matmul, and the 1/denom scaling is
folded into the PSUM->SBUF copy of o^T.  QKV biases are folded into the
projection matmuls as a K=1 accumulation row (bias x ones).

Pipeline: k/v projections streamed per s-group, then per sq-group {q-proj ->
scores -> exp*mask ->
P@V (software-pipelined one sk-chunk behind) -> normalize -> previous
group's output projection}, with PSUM budgeted exactly: 4 banks scores
(2x[128,1024] double-buffered, shared with q-proj and out-proj tiles) +
4 banks o^T accumulators.
"""

import sys

sys.path.insert(0, '/opt/trn_rl_repo')

import numpy as np

B, S, D = 2, 2048, 1024
H = 16
DK = 64
HC = 4            # heads per core
NC_ = HC * DK     # 256 projected dims per core
NCORES = 8

_cached = {}


def _build_nc():
    import concourse.bacc as bacc
    import concourse.mybir as mybir
    from concourse.tile import TileContext

    f32 = mybir.dt.float32
    f32r = mybir.dt.float32r
    bf16 = mybir.dt.bfloat16
    Exp = mybir.ActivationFunctionType.Exp

    nc = bacc.Bacc()

    XQT = nc.declare_dram_parameter("xqT", [D, S], bf16, isOutput=False)
    XKT = nc.declare_dram_parameter("xkT", [D, S], bf16, isOutput=False)
    XVT = nc.declare_dram_parameter("xvT", [D, S], bf16, isOutput=False)
    WQ = nc.declare_dram_parameter("wq", [D, NC_], bf16, isOutput=False)
    WK = nc.declare_dram_parameter("wk", [D, NC_], bf16, isOutput=False)
    WV = nc.declare_dram_parameter("wv", [D, NC_], bf16, isOutput=False)
    WO = nc.declare_dram_parameter("wo", [NC_, D], f32r, isOutput=False)
    BQ = nc.declare_dram_parameter("bq", [1, NC_], bf16, isOutput=False)
    BK = nc.declare_dram_parameter("bk", [1, NC_], bf16, isOutput=False)
    BV = nc.declare_dram_parameter("bv", [1, NC_], f32, isOutput=False)
    MT = nc.declare_dram_parameter("maskT", [S, S], bf16, isOutput=False)
    OUT = nc.declare_dram_parameter("out", [S, D], f32, isOutput=True)

    NT = NC_ // 128          # 2 n-tiles of 128 (pairs of heads)
    NDC = D // 128           # 8 d chunks
    NG = S // 512            # 4 sq groups
    NCk = S // 128           # 16 sk chunks
    NST = S // 128           # 16 s tiles

    with TileContext(nc) as tc:
        import contextlib
        ctx = contextlib.ExitStack()
        with ctx:
            consts = ctx.enter_context(tc.tile_pool(name="consts", bufs=1))
            xts = ctx.enter_context(tc.tile_pool(name="xts", bufs=1))
            pts = ctx.enter_context(tc.tile_pool(name="pts", bufs=2))
            mts = ctx.enter_context(tc.tile_pool(name="mts", bufs=2))
            smalls = ctx.enter_context(tc.tile_pool(name="smalls", bufs=3))
            outs = ctx.enter_context(tc.tile_pool(name="outs", bufs=4))

            # ---- constants ----
            wq_sb = consts.tile([128, NDC, NC_], bf16)
            wk_sb = consts.tile([128, NDC, NC_], bf16)
            wv_sb = consts.tile([128, NDC, NC_], bf16)
            wo_sb = consts.tile([128, NT, D], f32r)
            bq_sb = consts.tile([1, NC_], bf16)
            bk_sb = consts.tile([1, NC_], bf16)
            nc.sync.dma_start(out=wk_sb, in_=WK[:].rearrange("(c p) n -> p c n", p=128))
            nc.sync.dma_start(out=bk_sb, in_=BK[:])
            nc.sync.dma_start(out=wv_sb, in_=WV[:].rearrange("(c p) n -> p c n", p=128))
            nc.sync.dma_start(out=wq_sb, in_=WQ[:].rearrange("(c p) n -> p c n", p=128))
            nc.sync.dma_start(out=bq_sb, in_=BQ[:])
            nc.sync.dma_start(out=wo_sb, in_=WO[:].rearrange("(c p) n -> p c n", p=128))
            ones512 = consts.tile([1, 512], bf16)
            nc.vector.memset(ones512, 1.0)
            bv_row = consts.tile([1, NC_], f32)
            nc.sync.dma_start(out=bv_row, in_=BV[:])
            bv_bc = consts.tile([128, NC_], f32)
            nc.gpsimd.partition_broadcast(bv_bc, bv_row)
            ones_col = consts.tile([128, HC], bf16)
            nc.vector.memset(ones_col, 1.0)

            qT = [consts.tile([128, S], f32r, tag=f"qT{i}", name=f"qT{i}") for i in range(NT)]
            kT = [consts.tile([128, S], f32r, tag=f"kT{i}", name=f"kT{i}") for i in range(NT)]
            v_aug = consts.tile([128, NST, HC * 65], bf16)
            oTn = [consts.tile([128, S], f32r, tag=f"oTn{i}", name=f"oTn{i}") for i in range(NT)]

            # prefetch attention-critical-path data ahead of the bulk k/v
            # input stream: q group 0 and the first mask tiles
            xq_pre = xts.tile([128, NDC, 512], bf16, tag="xq", name="xq0", bufs=2)
            nc.sync.dma_start(
                out=xq_pre,
                in_=XQT[:, 0:512].rearrange("(c p) n -> p c n", p=128))

            # ---- phase 1: k/v projections (per-s-group streaming, double-
            # buffered; attention consumes all of kT/v_aug, so these come
            # first; q streams per-group into the attention loop below). ----
            with tc.tile_pool(name="pp", bufs=4, space="PSUM") as pp:
                for g in range(NG):
                    # k projection for this s-group
                    xgk = xts.tile([128, NDC, 512], bf16, tag="xk", name=f"xk{g}", bufs=2)
                    nc.sync.dma_start(
                        out=xgk,
                        in_=XKT[:, g * 512:(g + 1) * 512].rearrange("(c p) n -> p c n", p=128))
                    for nt in range(NT):
                        ps = pp.tile([128, 512], f32)
                        for dc in range(NDC):
                            nc.tensor.matmul(
                                ps[:],
                                wk_sb[:, dc, nt * 128:(nt + 1) * 128],
                                xgk[:, dc, :],
                                start=(dc == 0), stop=False,
                            )
                        nc.tensor.matmul(
                            ps[:],
                            bk_sb[0:1, nt * 128:(nt + 1) * 128],
                            ones512[0:1, :],
                            start=False, stop=True,
                        )
                        nc.vector.tensor_copy(
                            kT[nt][:, g * 512:(g + 1) * 512], ps[:])
                    # v projection for the same s-group (natural [s, n] layout)
                    xgv = xts.tile([128, NDC, 512], bf16, tag="xv", name=f"xv{g}", bufs=2)
                    nc.sync.dma_start(
                        out=xgv,
                        in_=XVT[:, g * 512:(g + 1) * 512].rearrange("(c p) n -> p c n", p=128))
                    for sl in range(4):
                        st = 4 * g + sl
                        ps = pp.tile([128, 512], f32)
                        for dc in range(NDC):
                            nc.tensor.matmul(
                                ps[:, 0:NC_],
                                xgv[:, dc, sl * 128:(sl + 1) * 128],
                                wv_sb[:, dc, :],
                                start=(dc == 0), stop=(dc == NDC - 1),
                            )
                        for h in range(HC):
                            nc.vector.tensor_add(
                                out=v_aug[:, st, h * 65:h * 65 + 64],
                                in0=ps[:, h * 64:(h + 1) * 64],
                                in1=bv_bc[:, h * 64:(h + 1) * 64],
                            )
                        nc.vector.tensor_copy(
                            v_aug.rearrange("p s (h c) -> p s h c", c=65)[:, st, :, 64],
                            ones_col[:],
                        )
            # ---- phase 2: q projection + attention, streamed per sq group ----
            import concourse.bass as bass
            with tc.tile_pool(name="sp", bufs=2, space="PSUM") as sp, \
                 tc.tile_pool(name="op", bufs=1, space="PSUM") as op:

                # prefetch the first few mask tiles so attention g=0 can
                # start exp/mul as soon as scores land
                mt_pre = {}
                for c in range(3):
                    base = MT[c * 128:(c + 1) * 128, 0:512]
                    mrep = bass.AP(tensor=base.tensor, offset=base.offset,
                                   ap=[base.ap[0], [0, 2], base.ap[1]])
                    mt = mts.tile([128, 2, 512], bf16, tag="mt", name=f"mtpre{c}", bufs=4)
                    nc.sync.dma_start(out=mt, in_=mrep)
                    mt_pre[(0, c)] = mt

                def emit_outproj_st(st):
                        fps = sp.tile([128, D], f32, tag="sps", name=f"fps{st}")
                        for kc in range(NT):
                            for dg in range(D // 512):
                                nc.tensor.matmul(
                                    fps[:, dg * 512:(dg + 1) * 512],
                                    oTn[kc][:, st * 128:(st + 1) * 128],
                                    wo_sb[:, kc, dg * 512:(dg + 1) * 512],
                                    start=(kc == 0), stop=(kc == NT - 1),
                                )
                        osb = outs.tile([128, D], f32)
                        nc.scalar.activation(osb[:], fps[:], mybir.ActivationFunctionType.Copy)
                        nc.sync.dma_start(out=OUT[st * 128:(st + 1) * 128, :], in_=osb[:])

                for g in range(NG):
                    # q projection for this group (PSUM from the shared sp pool)
                    if g == 0:
                        xg = xq_pre
                    else:
                        xg = xts.tile([128, NDC, 512], bf16, tag="xq", name=f"xq{g}", bufs=2)
                        nc.sync.dma_start(
                            out=xg,
                            in_=XQT[:, g * 512:(g + 1) * 512].rearrange("(c p) n -> p c n", p=128))
                    for nt in range(NT):
                        ps = sp.tile([128, D], f32, tag="sps", name=f"qps{g}_{nt}")
                        for dc in range(NDC):
                            nc.tensor.matmul(
                                ps[:, 0:512],
                                wq_sb[:, dc, nt * 128:(nt + 1) * 128],
                                xg[:, dc, :],
                                start=(dc == 0), stop=False,
                            )
                        nc.tensor.matmul(
                            ps[:, 0:512],
                            bq_sb[0:1, nt * 128:(nt + 1) * 128],
                            ones512[0:1, :],
                            start=False, stop=True,
                        )
                        nc.vector.tensor_copy(
                            qT[nt][:, g * 512:(g + 1) * 512], ps[:, 0:512])

                    ot = [op.tile([65, 512], f32, tag=f"ot{h}", name=f"ot{h}_{g}") for h in range(HC)]

                    def emit_v(c, pts_pair, ot=ot):
                        for p in range(2):
                            for half in range(2):
                                h = 2 * p + half
                                nc.tensor.matmul(
                                    ot[h][:],
                                    v_aug[:, c, h * 65:(h + 1) * 65],
                                    pts_pair[p][:, half * 512:(half + 1) * 512],
                                    start=(c == 0), stop=(c == NCk - 1),
                                )

                    prev = None
                    for c in range(NCk):
                        mt = mt_pre.pop((g, c), None)
                        if mt is None:
                            base = MT[c * 128:(c + 1) * 128, g * 512:(g + 1) * 512]
                            mrep = bass.AP(tensor=base.tensor, offset=base.offset,
                                           ap=[base.ap[0], [0, 2], base.ap[1]])
                            mt = mts.tile([128, 2, 512], bf16, tag="mt", name=f"mt{g}_{c}", bufs=4)
                            nc.sync.dma_start(out=mt, in_=mrep)
                        mt_flat = mt[:].rearrange("p a b -> p (a b)")
                        cur = []
                        for p in range(2):
                            sps = sp.tile([128, 1024], f32, tag="sps", name=f"sps{g}_{c}_{p}")
                            for half in range(2):
                                nc.tensor.matmul(
                                    sps[:, half * 512:(half + 1) * 512],
                                    kT[p][half * 64:half * 64 + 64, c * 128:(c + 1) * 128],
                                    qT[p][half * 64:half * 64 + 64, g * 512:(g + 1) * 512],
                                    start=True, stop=True,
                                )
                            pt = pts.tile([128, 1024], bf16, tag=f"pt{p}", name=f"pt{p}_{g}_{c}", bufs=3)
                            nc.scalar.activation(pt[:], sps[:], Exp, scale=0.125)
                            nc.vector.tensor_mul(pt[:], pt[:], mt_flat)
                            cur.append(pt)
                        if prev is not None:
                            emit_v(c - 1, prev)
                        prev = cur
                    c = NCk - 1
                    for p in range(2):
                        for half in range(2):
                            h = 2 * p + half
                            nc.tensor.matmul(
                                ot[h][:],
                                v_aug[:, c, h * 65:(h + 1) * 65],
                                prev[p][:, half * 512:(half + 1) * 512],
                                start=(c == 0), stop=True,
                            )
                            rc = smalls.tile([1, 512], f32, tag="rc", name=f"rc{g}_{h}")
                            nc.vector.reciprocal(rc, ot[h][64:65, :])
                            rb = smalls.tile([64, 512], f32, tag="rb", name=f"rb{g}_{h}")
                            nc.gpsimd.partition_broadcast(rb, rc)
                            nc.vector.tensor_mul(
                                oTn[p][half * 64:half * 64 + 64, g * 512:(g + 1) * 512],
                                ot[h][0:64, :], rb[:],
                            )
                    # deferred output projection: previous group's s-tiles
                    if g > 0:
                        for st in range(4 * (g - 1), 4 * g - 4 + 4):
                            emit_outproj_st(st)
                for st in range(4 * (NG - 1), 4 * NG):
                    emit_outproj_st(st)

    nc.compile()
    return nc


def _get_nc():
    if "nc" not in _cached:
        _cached["nc"] = _build_nc()
    return _cached["nc"]


def _make_in_maps(inputs):
    queries = np.asarray(inputs["queries"], dtype=np.float32)
    keys = np.asarray(inputs["keys"], dtype=np.float32)
    values = np.asarray(inputs["values"], dtype=np.float32)
    Wq = np.asarray(inputs["Wq"], dtype=np.float32)
    Wk = np.asarray(inputs["Wk"], dtype=np.float32)
    Wv = np.asarray(inputs["Wv"], dtype=np.float32)
    Wo = np.asarray(inputs["Wo"], dtype=np.float32)
    bq = np.asarray(inputs["bq"], dtype=np.float32)
    bk = np.asarray(inputs["bk"], dtype=np.float32)
    bv = np.asarray(inputs["bv"], dtype=np.float32)
    mask = np.asarray(inputs["mask"])

    import ml_dtypes
    bf = ml_dtypes.bfloat16
    xqT = [np.ascontiguousarray(queries[b].T.astype(bf)) for b in range(B)]
    xkT = [np.ascontiguousarray(keys[b].T.astype(bf)) for b in range(B)]
    xvT = [np.ascontiguousarray(values[b].T.astype(bf)) for b in range(B)]
    maskT = [np.ascontiguousarray(mask[b, 0].T.astype(bf)) for b in range(B)]

    in_maps = []
    for c in range(NCORES):
        b = c // 4
        h0 = (c % 4) * HC
        sl = slice(h0 * DK, (h0 + HC) * DK)
        in_maps.append({
            "xqT": xqT[b], "xkT": xkT[b], "xvT": xvT[b],
            "wq": np.ascontiguousarray(Wq[:, sl].astype(bf)),
            "wk": np.ascontiguousarray(Wk[:, sl].astype(bf)),
            "wv": np.ascontiguousarray(Wv[:, sl].astype(bf)),
            "wo": np.ascontiguousarray(Wo[sl, :]),
            "bq": np.ascontiguousarray(bq[sl].reshape(1, NC_).astype(bf)),
            "bk": np.ascontiguousarray(bk[sl].reshape(1, NC_).astype(bf)),
            "bv": np.ascontiguousarray(bv[sl].reshape(1, NC_)),
            "maskT": maskT[b],
        })
    return in_maps


def _combine(results, bo):
    out = np.empty((B, S, D), dtype=np.float32)
    for b in range(B):
        acc = results[4 * b]["out"].astype(np.float32).copy()
        for c in range(4 * b + 1, 4 * b + 4):
            acc += results[c]["out"]
        out[b] = acc + bo[None, :]
    return out


def kernel(queries, keys, values, Wq, bq, Wk, bk, Wv, bv, Wo, bo, mask):
    from concourse.bass_utils import run_bass_kernel_spmd

    nc = _get_nc()
    in_maps = _make_in_maps(dict(
        queries=queries, keys=keys, values=values, Wq=Wq, Wk=Wk, Wv=Wv, Wo=Wo,
        bq=bq, bk=bk, bv=bv, mask=mask))
    res = run_bass_kernel_spmd(nc, in_maps, list(range(NCORES)))
    return _combine(res.results, np.asarray(bo, dtype=np.float32))


# revision 32
# speedup vs baseline: 1.0124x; 1.0124x over previous
"""MultiHeadAttention Trainium2 kernel.

Sharding: B=2 batches x H=16 heads = 32 (b,h) pairs -> 4 heads per core.
Cores 0-3 handle batch 0 (heads 4c..4c+3), cores 4-7 batch 1.
Each core computes q/k/v projections for its head slice, transposed-scores
attention, and a partial output projection (sum over its heads of
o_h @ Wo[h-slice]).  Host sums the 4 partials per batch and adds bo.

Projection inputs/weights are bf16 (halves input DMA); scores operands
(qT/kT), oTn and Wo are float32r (rounded fp32, 1 cycle/row on PE at N>=256);
post-softmax probabilities and V are bf16.  Softmax skips max-subtraction
(scores ~ N(0,1); exp cannot overflow) and is computed in the transposed
layout PT[sk, sq] = exp(scores^T) * mask^T; denominators come from a
ones-column appended to V in the P@V matmul, and the 1/denom scaling is
folded into the PSUM->SBUF copy of o^T.  QKV biases are folded into the
projection matmuls as a K=1 accumulation row (bias x ones).

Pipeline: k/v projections streamed per s-group, then per sq-group {q-proj ->
scores -> exp*mask ->
P@V (software-pipelined one sk-chunk behind) -> normalize -> previous
group's output projection}, with PSUM budgeted exactly: 4 banks scores
(2x[128,1024] double-buffered, shared with q-proj and out-proj tiles) +
4 banks o^T accumulators.
"""

import sys

sys.path.insert(0, '/opt/trn_rl_repo')

import numpy as np

B, S, D = 2, 2048, 1024
H = 16
DK = 64
HC = 4            # heads per core
NC_ = HC * DK     # 256 projected dims per core
NCORES = 8

_cached = {}


def _build_nc():
    import concourse.bacc as bacc
    import concourse.mybir as mybir
    from concourse.tile import TileContext

    f32 = mybir.dt.float32
    f32r = mybir.dt.float32r
    bf16 = mybir.dt.bfloat16
    Exp = mybir.ActivationFunctionType.Exp

    nc = bacc.Bacc()

    XQT = nc.declare_dram_parameter("xqT", [D, S], bf16, isOutput=False)
    XKT = nc.declare_dram_parameter("xkT", [D, S], bf16, isOutput=False)
    XVT = nc.declare_dram_parameter("xvT", [D, S], bf16, isOutput=False)
    WQ = nc.declare_dram_parameter("wq", [D, NC_], bf16, isOutput=False)
    WK = nc.declare_dram_parameter("wk", [D, NC_], bf16, isOutput=False)
    WV = nc.declare_dram_parameter("wv", [D, NC_], bf16, isOutput=False)
    WO = nc.declare_dram_parameter("wo", [NC_, D], f32r, isOutput=False)
    BQ = nc.declare_dram_parameter("bq", [1, NC_], bf16, isOutput=False)
    BK = nc.declare_dram_parameter("bk", [1, NC_], bf16, isOutput=False)
    BV = nc.declare_dram_parameter("bv", [1, NC_], f32, isOutput=False)
    MT = nc.declare_dram_parameter("maskT", [S, S], bf16, isOutput=False)
    OUT = nc.declare_dram_parameter("out", [S, D], f32, isOutput=True)

    NT = NC_ // 128          # 2 n-tiles of 128 (pairs of heads)
    NDC = D // 128           # 8 d chunks
    NG = S // 512            # 4 sq groups
    NCk = S // 128           # 16 sk chunks
    NST = S // 128           # 16 s tiles

    with TileContext(nc) as tc:
        import contextlib
        ctx = contextlib.ExitStack()
        with ctx:
            consts = ctx.enter_context(tc.tile_pool(name="consts", bufs=1))
            xts = ctx.enter_context(tc.tile_pool(name="xts", bufs=1))
            pts = ctx.enter_context(tc.tile_pool(name="pts", bufs=2))
            mts = ctx.enter_context(tc.tile_pool(name="mts", bufs=2))
            smalls = ctx.enter_context(tc.tile_pool(name="smalls", bufs=3))
            outs = ctx.enter_context(tc.tile_pool(name="outs", bufs=4))

            # ---- constants ----
            wq_sb = consts.tile([128, NDC, NC_], bf16)
            wk_sb = consts.tile([128, NDC, NC_], bf16)
            wv_sb = consts.tile([128, NDC, NC_], bf16)
            wo_sb = consts.tile([128, NT, D], f32r)
            bq_sb = consts.tile([1, NC_], bf16)
            bk_sb = consts.tile([1, NC_], bf16)
            nc.sync.dma_start(out=wk_sb, in_=WK[:].rearrange("(c p) n -> p c n", p=128))
            nc.sync.dma_start(out=bk_sb, in_=BK[:])
            nc.sync.dma_start(out=wv_sb, in_=WV[:].rearrange("(c p) n -> p c n", p=128))
            nc.sync.dma_start(out=wq_sb, in_=WQ[:].rearrange("(c p) n -> p c n", p=128))
            nc.sync.dma_start(out=bq_sb, in_=BQ[:])
            nc.sync.dma_start(out=wo_sb, in_=WO[:].rearrange("(c p) n -> p c n", p=128))
            ones512 = consts.tile([1, 512], bf16)
            nc.vector.memset(ones512, 1.0)
            bv_row = consts.tile([1, NC_], f32)
            nc.sync.dma_start(out=bv_row, in_=BV[:])
            bv_bc = consts.tile([128, NC_], f32)
            nc.gpsimd.partition_broadcast(bv_bc, bv_row)
            ones_col = consts.tile([128, HC], bf16)
            nc.vector.memset(ones_col, 1.0)

            qT = [consts.tile([128, S], f32r, tag=f"qT{i}", name=f"qT{i}") for i in range(NT)]
            kT = [consts.tile([128, S], f32r, tag=f"kT{i}", name=f"kT{i}") for i in range(NT)]
            v_aug = consts.tile([128, NST, HC * 65], bf16)
            oTn = [consts.tile([128, S], f32r, tag=f"oTn{i}", name=f"oTn{i}") for i in range(NT)]

            # ---- phase 1: k/v projections (per-s-group streaming, double-
            # buffered; attention consumes all of kT/v_aug, so these come
            # first; q streams per-group into the attention loop below). ----
            with tc.tile_pool(name="pp", bufs=4, space="PSUM") as pp:
                for g in range(NG):
                    # k projection for this s-group
                    xgk = xts.tile([128, NDC, 512], bf16, tag="xk", name=f"xk{g}", bufs=2)
                    nc.sync.dma_start(
                        out=xgk,
                        in_=XKT[:, g * 512:(g + 1) * 512].rearrange("(c p) n -> p c n", p=128))
                    for nt in range(NT):
                        ps = pp.tile([128, 512], f32)
                        for dc in range(NDC):
                            nc.tensor.matmul(
                                ps[:],
                                wk_sb[:, dc, nt * 128:(nt + 1) * 128],
                                xgk[:, dc, :],
                                start=(dc == 0), stop=False,
                            )
                        nc.tensor.matmul(
                            ps[:],
                            bk_sb[0:1, nt * 128:(nt + 1) * 128],
                            ones512[0:1, :],
                            start=False, stop=True,
                        )
                        nc.vector.tensor_copy(
                            kT[nt][:, g * 512:(g + 1) * 512], ps[:])
                    # v projection for the same s-group (natural [s, n] layout)
                    xgv = xts.tile([128, NDC, 512], bf16, tag="xv", name=f"xv{g}", bufs=2)
                    nc.sync.dma_start(
                        out=xgv,
                        in_=XVT[:, g * 512:(g + 1) * 512].rearrange("(c p) n -> p c n", p=128))
                    for sl in range(4):
                        st = 4 * g + sl
                        ps = pp.tile([128, 512], f32)
                        for dc in range(NDC):
                            nc.tensor.matmul(
                                ps[:, 0:NC_],
                                xgv[:, dc, sl * 128:(sl + 1) * 128],
                                wv_sb[:, dc, :],
                                start=(dc == 0), stop=(dc == NDC - 1),
                            )
                        for h in range(HC):
                            nc.vector.tensor_add(
                                out=v_aug[:, st, h * 65:h * 65 + 64],
                                in0=ps[:, h * 64:(h + 1) * 64],
                                in1=bv_bc[:, h * 64:(h + 1) * 64],
                            )
                        nc.vector.tensor_copy(
                            v_aug.rearrange("p s (h c) -> p s h c", c=65)[:, st, :, 64],
                            ones_col[:],
                        )
            # ---- phase 2: q projection + attention, streamed per sq group ----
            import concourse.bass as bass
            with tc.tile_pool(name="sp", bufs=2, space="PSUM") as sp, \
                 tc.tile_pool(name="op", bufs=1, space="PSUM") as op:

                # prefetch the first few mask tiles so attention g=0 can
                # start exp/mul as soon as scores land
                mt_pre = {}
                for c in range(3):
                    base = MT[c * 128:(c + 1) * 128, 0:512]
                    mrep = bass.AP(tensor=base.tensor, offset=base.offset,
                                   ap=[base.ap[0], [0, 2], base.ap[1]])
                    mt = mts.tile([128, 2, 512], bf16, tag="mt", name=f"mtpre{c}", bufs=4)
                    nc.sync.dma_start(out=mt, in_=mrep)
                    mt_pre[(0, c)] = mt

                def emit_outproj_st(st):
                        fps = sp.tile([128, D], f32, tag="sps", name=f"fps{st}")
                        for kc in range(NT):
                            for dg in range(D // 512):
                                nc.tensor.matmul(
                                    fps[:, dg * 512:(dg + 1) * 512],
                                    oTn[kc][:, st * 128:(st + 1) * 128],
                                    wo_sb[:, kc, dg * 512:(dg + 1) * 512],
                                    start=(kc == 0), stop=(kc == NT - 1),
                                )
                        osb = outs.tile([128, D], f32)
                        nc.scalar.activation(osb[:], fps[:], mybir.ActivationFunctionType.Copy)
                        nc.sync.dma_start(out=OUT[st * 128:(st + 1) * 128, :], in_=osb[:])

                for g in range(NG):
                    # q projection for this group (PSUM from the shared sp pool)
                    xg = xts.tile([128, NDC, 512], bf16, tag="xq", name=f"xq{g}", bufs=2)
                    nc.sync.dma_start(
                        out=xg,
                        in_=XQT[:, g * 512:(g + 1) * 512].rearrange("(c p) n -> p c n", p=128))
                    for nt in range(NT):
                        ps = sp.tile([128, D], f32, tag="sps", name=f"qps{g}_{nt}")
                        for dc in range(NDC):
                            nc.tensor.matmul(
                                ps[:, 0:512],
                                wq_sb[:, dc, nt * 128:(nt + 1) * 128],
                                xg[:, dc, :],
                                start=(dc == 0), stop=False,
                            )
                        nc.tensor.matmul(
                            ps[:, 0:512],
                            bq_sb[0:1, nt * 128:(nt + 1) * 128],
                            ones512[0:1, :],
                            start=False, stop=True,
                        )
                        nc.vector.tensor_copy(
                            qT[nt][:, g * 512:(g + 1) * 512], ps[:, 0:512])

                    ot = [op.tile([65, 512], f32, tag=f"ot{h}", name=f"ot{h}_{g}") for h in range(HC)]

                    def emit_v(c, pts_pair, ot=ot):
                        for p in range(2):
                            for half in range(2):
                                h = 2 * p + half
                                nc.tensor.matmul(
                                    ot[h][:],
                                    v_aug[:, c, h * 65:(h + 1) * 65],
                                    pts_pair[p][:, half * 512:(half + 1) * 512],
                                    start=(c == 0), stop=(c == NCk - 1),
                                )

                    prev = None
                    for c in range(NCk):
                        mt = mt_pre.pop((g, c), None)
                        if mt is None:
                            base = MT[c * 128:(c + 1) * 128, g * 512:(g + 1) * 512]
                            mrep = bass.AP(tensor=base.tensor, offset=base.offset,
                                           ap=[base.ap[0], [0, 2], base.ap[1]])
                            mt = mts.tile([128, 2, 512], bf16, tag="mt", name=f"mt{g}_{c}", bufs=4)
                            nc.sync.dma_start(out=mt, in_=mrep)
                        mt_flat = mt[:].rearrange("p a b -> p (a b)")
                        cur = []
                        for p in range(2):
                            sps = sp.tile([128, 1024], f32, tag="sps", name=f"sps{g}_{c}_{p}")
                            for half in range(2):
                                nc.tensor.matmul(
                                    sps[:, half * 512:(half + 1) * 512],
                                    kT[p][half * 64:half * 64 + 64, c * 128:(c + 1) * 128],
                                    qT[p][half * 64:half * 64 + 64, g * 512:(g + 1) * 512],
                                    start=True, stop=True,
                                )
                            pt = pts.tile([128, 1024], bf16, tag=f"pt{p}", name=f"pt{p}_{g}_{c}", bufs=3)
                            nc.scalar.activation(pt[:], sps[:], Exp, scale=0.125)
                            nc.vector.tensor_mul(pt[:], pt[:], mt_flat)
                            cur.append(pt)
                        if prev is not None:
                            emit_v(c - 1, prev)
                        prev = cur
                    c = NCk - 1
                    for p in range(2):
                        for half in range(2):
                            h = 2 * p + half
                            nc.tensor.matmul(
                                ot[h][:],
                                v_aug[:, c, h * 65:(h + 1) * 65],
                                prev[p][:, half * 512:(half + 1) * 512],
                                start=(c == 0), stop=True,
                            )
                            rc = smalls.tile([1, 512], f32, tag="rc", name=f"rc{g}_{h}")
                            nc.vector.reciprocal(rc, ot[h][64:65, :])
                            rb = smalls.tile([64, 512], f32, tag="rb", name=f"rb{g}_{h}")
                            nc.gpsimd.partition_broadcast(rb, rc)
                            nc.vector.tensor_mul(
                                oTn[p][half * 64:half * 64 + 64, g * 512:(g + 1) * 512],
                                ot[h][0:64, :], rb[:],
                            )
                    # deferred output projection: previous group's s-tiles
                    if g > 0:
                        for st in range(4 * (g - 1), 4 * g - 4 + 4):
                            emit_outproj_st(st)
                for st in range(4 * (NG - 1), 4 * NG):
                    emit_outproj_st(st)

    nc.compile()
    return nc


def _get_nc():
    if "nc" not in _cached:
        _cached["nc"] = _build_nc()
    return _cached["nc"]


def _make_in_maps(inputs):
    queries = np.asarray(inputs["queries"], dtype=np.float32)
    keys = np.asarray(inputs["keys"], dtype=np.float32)
    values = np.asarray(inputs["values"], dtype=np.float32)
    Wq = np.asarray(inputs["Wq"], dtype=np.float32)
    Wk = np.asarray(inputs["Wk"], dtype=np.float32)
    Wv = np.asarray(inputs["Wv"], dtype=np.float32)
    Wo = np.asarray(inputs["Wo"], dtype=np.float32)
    bq = np.asarray(inputs["bq"], dtype=np.float32)
    bk = np.asarray(inputs["bk"], dtype=np.float32)
    bv = np.asarray(inputs["bv"], dtype=np.float32)
    mask = np.asarray(inputs["mask"])

    import ml_dtypes
    bf = ml_dtypes.bfloat16
    xqT = [np.ascontiguousarray(queries[b].T.astype(bf)) for b in range(B)]
    xkT = [np.ascontiguousarray(keys[b].T.astype(bf)) for b in range(B)]
    xvT = [np.ascontiguousarray(values[b].T.astype(bf)) for b in range(B)]
    maskT = [np.ascontiguousarray(mask[b, 0].T.astype(bf)) for b in range(B)]

    in_maps = []
    for c in range(NCORES):
        b = c // 4
        h0 = (c % 4) * HC
        sl = slice(h0 * DK, (h0 + HC) * DK)
        in_maps.append({
            "xqT": xqT[b], "xkT": xkT[b], "xvT": xvT[b],
            "wq": np.ascontiguousarray(Wq[:, sl].astype(bf)),
            "wk": np.ascontiguousarray(Wk[:, sl].astype(bf)),
            "wv": np.ascontiguousarray(Wv[:, sl].astype(bf)),
            "wo": np.ascontiguousarray(Wo[sl, :]),
            "bq": np.ascontiguousarray(bq[sl].reshape(1, NC_).astype(bf)),
            "bk": np.ascontiguousarray(bk[sl].reshape(1, NC_).astype(bf)),
            "bv": np.ascontiguousarray(bv[sl].reshape(1, NC_)),
            "maskT": maskT[b],
        })
    return in_maps


def _combine(results, bo):
    out = np.empty((B, S, D), dtype=np.float32)
    for b in range(B):
        acc = results[4 * b]["out"].astype(np.float32).copy()
        for c in range(4 * b + 1, 4 * b + 4):
            acc += results[c]["out"]
        out[b] = acc + bo[None, :]
    return out


def kernel(queries, keys, values, Wq, bq, Wk, bk, Wv, bv, Wo, bo, mask):
    from concourse.bass_utils import run_bass_kernel_spmd

    nc = _get_nc()
    in_maps = _make_in_maps(dict(
        queries=queries, keys=keys, values=values, Wq=Wq, Wk=Wk, Wv=Wv, Wo=Wo,
        bq=bq, bk=bk, bv=bv, mask=mask))
    res = run_bass_kernel_spmd(nc, in_maps, list(range(NCORES)))
    return _combine(res.results, np.asarray(bo, dtype=np.float32))
